# revision 1
# baseline (speedup 1.0000x reference)
"""Trainium2 Bass kernel for nn_CrossAttentionGraphBlock.

Strategy (hardcoded for B=16, NQ=512, NK=1024, D=768, L=512, H=12, DH=64):
 - Data-parallel over batch: 8 cores x 2 batches each. No collectives.
 - Host-side (numpy, cheap): fuse the outer q/k/v projections with the
   in-proj of MultiheadAttention (both are linear):
       qWe = qW @ in_qW / sqrt(DH)   (softmax scale folded in)
       kWe = kW @ in_kW,  vWe = vW @ in_vW  (+ fused biases)
   and pre-transpose activations so the device never transposes inputs.
 - On-chip dataflow is feature-major ([feature, token] in SBUF) end to end:
   projections, attention, out-proj, both layernorms.  Per head h:
       S^T[k,q]  = (kh_h)^T.T @ qh_h^T          (K=DH=64 contraction)
       P^T       = exp(S^T)                      (no max-sub needed: |S|<~1)
       ctx~aug^T = vh_aug.T @ P^T                (65th row = softmax denom)
   then ctx^T rows are scaled by 1/denom (PE broadcast of the reciprocal).
   Key-padding mask is applied by zeroing masked rows of vh_aug (incl. the
   ones-column), which removes masked keys from both ctx and the denom.
 - Heads are processed in pairs with interleaved S matmuls at partition
   bases 0/64 -> tile_position row groups (0,0)/(64,0) run concurrently.
 - LayerNorm stats across features (= partitions) via ones-column matmuls.
 - Final [feat,tok] -> [tok,feat] transpose on the tensor engine.
 - Precision: matmuls bf16 (fp32 PSUM accumulate); LN stats, softmax
   denominators and the final LN2 output stay fp32.
"""

import numpy as np
import ml_dtypes

import concourse.bass as bass
import concourse.mybir as mybir
import concourse.tile as tile
from concourse import bacc
from concourse.bass_utils import run_bass_kernel_spmd

P = 128
B, NQ, NK, D, L, H = 16, 512, 1024, 768, 512, 12
DH = D // H          # 64
NCORES = 8
BL = B // NCORES     # 2 batches per core
EPS = 1e-5
F32 = mybir.dt.float32
BF16 = mybir.dt.bfloat16
NPBF = ml_dtypes.bfloat16
AF = mybir.ActivationFunctionType
ALU = mybir.AluOpType

KD = D // P          # 6 chunks of the D (=768) contraction/feature dim
KL = L // P          # 4 chunks of the L (=512) contraction dim
MQ = NQ // P         # 4 query-token tiles
MK = NK // P         # 8 key-token tiles
VA = H * (DH + 1)    # 780: vh augmented with one ones-column per head

_NC_CACHE = {}


def _build_nc():
    nc = bacc.Bacc()

    gnT = nc.declare_dram_parameter("gnT", [BL, D, NQ], BF16, isOutput=False)
    gnTf = nc.declare_dram_parameter("gnTf", [BL, D, NQ], F32, isOutput=False)
    condT = nc.declare_dram_parameter("condT", [BL, L, NK], BF16, isOutput=False)
    qWe = nc.declare_dram_parameter("qWe", [D, D], BF16, isOutput=False)
    kWe = nc.declare_dram_parameter("kWe", [L, D], BF16, isOutput=False)
    vWe = nc.declare_dram_parameter("vWe", [L, VA], BF16, isOutput=False)
    outW = nc.declare_dram_parameter("outW", [D, D], BF16, isOutput=False)
    d1W = nc.declare_dram_parameter("d1W", [D, D], BF16, isOutput=False)
    bcols = nc.declare_dram_parameter("bcols", [P, 8 * KD], F32, isOutput=False)
    vber = nc.declare_dram_parameter("vber", [1, VA], BF16, isOutput=False)
    vld = nc.declare_dram_parameter("vld", [P, BL * MK], F32, isOutput=False)
    onesr = nc.declare_dram_parameter("onesr", [1, P], F32, isOutput=False)
    onesrb = nc.declare_dram_parameter("onesrb", [1, P], BF16, isOutput=False)
    onescb = nc.declare_dram_parameter("onescb", [P, 1], BF16, isOutput=False)
    onescf = nc.declare_dram_parameter("onescf", [P, 1], F32, isOutput=False)
    ident = nc.declare_dram_parameter("ident", [P, P], F32, isOutput=False)
    out = nc.declare_dram_parameter("out", [BL, NQ, D], F32, isOutput=True)

    with tile.TileContext(nc) as tc:
        with (
            tc.tile_pool(name="const", bufs=1) as cst,
            tc.tile_pool(name="gnT", bufs=2) as gnp,
            tc.tile_pool(name="big", bufs=2) as big,      # condT / per-head P~ / LN sq
            tc.tile_pool(name="kh", bufs=2) as khp,
            tc.tile_pool(name="qh", bufs=2) as qhp,
            tc.tile_pool(name="vh", bufs=2) as vhp,
            tc.tile_pool(name="xres", bufs=1) as xrp,
            tc.tile_pool(name="yy", bufs=1) as yyp,
            tc.tile_pool(name="outp", bufs=1) as otp,
            tc.tile_pool(name="ctx", bufs=1) as ctp,
            tc.tile_pool(name="zz", bufs=2) as zzp,
            tc.tile_pool(name="small", bufs=1) as sml,
            tc.tile_pool(name="sp", bufs=3, space="PSUM") as spp,   # [P,1024] 2-bank
            tc.tile_pool(name="mm", bufs=2, space="PSUM") as mmp,   # [P,512]
        ):
            # ---- resident constants -------------------------------------
            qWe_sb = cst.tile([P, KD, D], BF16, tag="qWe")
            nc.sync.dma_start(qWe_sb[:], qWe[:].rearrange("(ko p) n -> p ko n", p=P))
            kWe_sb = cst.tile([P, KL, D], BF16, tag="kWe")
            nc.sync.dma_start(kWe_sb[:], kWe[:].rearrange("(ko p) n -> p ko n", p=P))
            vWe_sb = cst.tile([P, KL, VA], BF16, tag="vWe")
            nc.sync.dma_start(vWe_sb[:], vWe[:].rearrange("(ko p) n -> p ko n", p=P))
            bc_sb = cst.tile([P, 8 * KD], F32, tag="bcols")
            nc.sync.dma_start(bc_sb[:], bcols[:])
            vber_sb = cst.tile([1, VA], BF16, tag="vber")
            nc.sync.dma_start(vber_sb[:], vber[:])
            vld_sb = cst.tile([P, BL * MK], F32, tag="vld")
            nc.sync.dma_start(vld_sb[:], vld[:])
            onesr_sb = cst.tile([1, P], F32, tag="onesr")
            nc.sync.dma_start(onesr_sb[:], onesr[:])
            onesrb_sb = cst.tile([1, P], BF16, tag="onesrb")
            nc.sync.dma_start(onesrb_sb[:], onesrb[:])
            onescb_sb = cst.tile([P, 1], BF16, tag="onescb")
            nc.sync.dma_start(onescb_sb[:], onescb[:])
            onescf_sb = cst.tile([P, 1], F32, tag="onescf")
            nc.sync.dma_start(onescf_sb[:], onescf[:])
            id_sb = cst.tile([P, P], F32, tag="ident")
            nc.sync.dma_start(id_sb[:], ident[:])
            # batch-0 inputs next, so the first projections start early;
            # late-used weights (outW/d1W) load after.
            gnT0_sb = gnp.tile([P, KD, NQ], BF16, tag="gnT")
            nc.sync.dma_start(gnT0_sb[:], gnT[0].rearrange("(ko p) t -> p ko t", p=P))
            condT0_sb = big.tile([P, KL, NK], BF16, tag="big", name="condT0")
            nc.sync.dma_start(condT0_sb[:], condT[0].rearrange("(ko p) t -> p ko t", p=P))
            gnTf0_sb = zzp.tile([P, KD, NQ], F32, tag="zz")
            nc.sync.dma_start(gnTf0_sb[:], gnTf[0].rearrange("(ko p) t -> p ko t", p=P))
            outW_sb = cst.tile([P, KD, D], BF16, tag="outW")
            nc.sync.dma_start(outW_sb[:], outW[:].rearrange("(ko p) n -> p ko n", p=P))
            d1W_sb = cst.tile([P, KD, D], BF16, tag="d1W")
            nc.sync.dma_start(d1W_sb[:], d1W[:].rearrange("(ko p) n -> p ko n", p=P))

            qbe_c = bc_sb[:, 0 * KD:1 * KD]
            kbe_c = bc_sb[:, 1 * KD:2 * KD]
            outb_c = bc_sb[:, 2 * KD:3 * KD]
            d1b_c = bc_sb[:, 3 * KD:4 * KD]
            ln1g_c = bc_sb[:, 4 * KD:5 * KD]
            ln1b_c = bc_sb[:, 5 * KD:6 * KD]
            ln2g_c = bc_sb[:, 6 * KD:7 * KD]
            ln2b_c = bc_sb[:, 7 * KD:8 * KD]

            def layer_norm(x_sb, g_c, b_c, out_sb):
                """Feature-major LN over partitions (768 feats = 6 chunks).
                Stats fp32; dtypes follow x_sb."""
                fp = x_sb.dtype == F32
                ones = onescf_sb if fp else onescb_sb
                sum1 = spp.tile([P, NK], F32, tag="sp", name="sum1")
                sq_sb = big.tile([P, KD, NQ], BF16, tag="big")
                sum2 = spp.tile([P, NK], F32, tag="sp", name="sum2")
                for kc in range(KD):
                    nc.tensor.matmul(sum1[0:1, :NQ], ones[:], x_sb[:, kc, :],
                                     start=(kc == 0), stop=(kc == KD - 1))
                    nc.scalar.activation(sq_sb[:, kc, :], x_sb[:, kc, :], AF.Square)
                    nc.tensor.matmul(sum2[0:1, :NQ], onescb_sb[:], sq_sb[:, kc, :],
                                     start=(kc == 0), stop=(kc == KD - 1))
                m_sb = sml.tile([1, NQ], F32, tag="m")
                nc.vector.tensor_scalar_mul(m_sb[:], sum1[0:1, :NQ], 1.0 / D)
                e2_sb = sml.tile([1, NQ], F32, tag="e2")
                nc.vector.tensor_scalar(e2_sb[:], sum2[0:1, :NQ], 1.0 / D, EPS,
                                        ALU.mult, ALU.add)
                msq_sb = sml.tile([1, NQ], F32, tag="msq_dtmp", name="msq_sb")
                nc.vector.tensor_tensor(msq_sb[:], m_sb[:], m_sb[:], ALU.mult)
                nc.vector.tensor_tensor(e2_sb[:], e2_sb[:], msq_sb[:], ALU.subtract)
                sd_sb = sml.tile([1, NQ], F32, tag="sd_rtmp", name="sd_sb")
                nc.scalar.activation(sd_sb[:], e2_sb[:], AF.Sqrt)
                rs_sb = sml.tile([1, NQ], F32, tag="rs_lnt", name="rs_sb")
                nc.vector.reciprocal(rs_sb[:], sd_sb[:])
                m_bc = spp.tile([P, NK], F32, tag="sp", name="m_bc")
                nc.tensor.matmul(m_bc[:, :NQ], onesr_sb[:], m_sb[:], start=True, stop=True)
                rs_bc = spp.tile([P, NK], F32, tag="sp", name="rs_bc")
                nc.tensor.matmul(rs_bc[:, :NQ], onesr_sb[:], rs_sb[:], start=True, stop=True)
                for kc in range(KD):
                    t_sb = sml.tile([P, NQ], F32, tag="rs_lnt", name="t_sb")
                    nc.vector.tensor_tensor(t_sb[:], x_sb[:, kc, :], m_bc[:, :NQ], ALU.subtract)
                    nc.vector.tensor_tensor(t_sb[:], t_sb[:], rs_bc[:, :NQ], ALU.mult)
                    nc.vector.tensor_scalar(out_sb[:, kc, :], t_sb[:],
                                            g_c[:, kc:kc + 1], b_c[:, kc:kc + 1],
                                            ALU.mult, ALU.add)

            for b in range(BL):
                # ---- input DMAs ----------------------------------------
                if b == 0:
                    gnT_sb, gnTf_sb, condT_sb = gnT0_sb, gnTf0_sb, condT0_sb
                else:
                    gnT_sb = gnp.tile([P, KD, NQ], BF16, tag="gnT")
                    nc.sync.dma_start(gnT_sb[:], gnT[b].rearrange("(ko p) t -> p ko t", p=P))
                    gnTf_sb = zzp.tile([P, KD, NQ], F32, tag="zz")
                    nc.sync.dma_start(gnTf_sb[:], gnTf[b].rearrange("(ko p) t -> p ko t", p=P))
                    condT_sb = big.tile([P, KL, NK], BF16, tag="big")
                    nc.sync.dma_start(condT_sb[:], condT[b].rearrange("(ko p) t -> p ko t", p=P))

                # ---- projections (feature-major, bf16 outputs) ----------
                qhT_sb = qhp.tile([P, KD, NQ], BF16, tag="qh")
                for m in range(KD):
                    ps = mmp.tile([P, 512], F32, tag="mm")
                    for kc in range(KD):
                        nc.tensor.matmul(ps[:, :NQ], qWe_sb[:, kc, m * P:(m + 1) * P],
                                         gnT_sb[:, kc, :], start=(kc == 0), stop=(kc == KD - 1))
                    nc.vector.tensor_scalar_add(qhT_sb[:, m, :], ps[:, :NQ], qbe_c[:, m:m + 1])

                khT_sb = khp.tile([P, KD, NK], BF16, tag="kh")
                for m in range(KD):
                    ps = spp.tile([P, NK], F32, tag="sp")
                    for n in range(2):
                        for kc in range(KL):
                            nc.tensor.matmul(ps[:, n * 512:(n + 1) * 512],
                                             kWe_sb[:, kc, m * P:(m + 1) * P],
                                             condT_sb[:, kc, n * 512:(n + 1) * 512],
                                             start=(kc == 0), stop=(kc == KL - 1))
                    nc.vector.tensor_scalar_add(khT_sb[:, m, :], ps[:], kbe_c[:, m:m + 1])

                vh_sb = vhp.tile([P, MK, VA], BF16, tag="vh")
                for mk in range(MK):
                    ps = spp.tile([P, NK], F32, tag="sp")
                    for (n0, nw) in ((0, 512), (512, VA - 512)):
                        for kc in range(KL):
                            nc.tensor.matmul(ps[:, n0:n0 + nw],
                                             condT_sb[:, kc, mk * P:(mk + 1) * P],
                                             vWe_sb[:, kc, n0:n0 + nw],
                                             start=(kc == 0), stop=False)
                        nc.tensor.matmul(ps[:, n0:n0 + nw], onesrb_sb[:],
                                         vber_sb[:, n0:n0 + nw], start=False, stop=True)
                    # bias included; now zero masked key rows (incl ones-col)
                    nc.scalar.activation(vh_sb[:, mk, :], ps[:, :VA], AF.Copy,
                                         scale=vld_sb[:, b * MK + mk: b * MK + mk + 1])

                # ---- attention (head pairs, feature-major) --------------
                ctxT_sb = ctp.tile([P, KD, NQ], BF16, tag="ctx")
                def s_block(hp, pT):
                    th = hp
                    for g2 in range(MK // 2):
                        s_ps = [spp.tile([P, NK], F32, tag="sp", name=f"s_ps{e}") for e in range(2)]
                        for half in range(2):
                            mk = 2 * g2 + half
                            for e in range(2):   # even/odd head interleaved
                                off = DH * e
                                nc.tensor.matmul(s_ps[e][:, half * NQ:(half + 1) * NQ],
                                                 khT_sb[off:off + DH, th, mk * P:(mk + 1) * P],
                                                 qhT_sb[off:off + DH, th, :],
                                                 start=True, stop=True)
                        for e in range(2):
                            nc.scalar.activation(pT[e][:, 2 * g2:2 * g2 + 2, :], s_ps[e][:], AF.Exp)

                for hp in range(H // 2):
                    th = hp
                    pT = [big.tile([P, MK, NQ], BF16, tag="big", name=f"pT{e}") for e in range(2)]
                    s_block(hp, pT)
                    for e in range(2):
                        h = 2 * hp + e
                        off = DH * e
                        c_ps = mmp.tile([P, 512], F32, tag="mm")
                        for kc in range(MK):
                            nc.tensor.matmul(c_ps[0:DH + 1, :NQ],
                                             vh_sb[:, kc, h * (DH + 1):(h + 1) * (DH + 1)],
                                             pT[e][:, kc, :],
                                             start=(kc == 0), stop=(kc == MK - 1))
                        # softmax denominator -> reciprocal -> PE broadcast
                        dtmp = sml.tile([1, NQ], F32, tag="msq_dtmp", name="dtmp")
                        nc.scalar.copy(dtmp[:], c_ps[DH:DH + 1, :NQ])
                        rtmp = sml.tile([1, NQ], F32, tag="sd_rtmp", name="rtmp")
                        nc.vector.reciprocal(rtmp[:], dtmp[:])
                        r_ps = mmp.tile([P, 512], F32, tag="mm")
                        nc.tensor.matmul(r_ps[0:DH, :NQ], onesr_sb[0:1, 0:DH], rtmp[:],
                                         start=True, stop=True)
                        nc.vector.tensor_copy(ctxT_sb[off:off + DH, th, :], c_ps[0:DH, :NQ])
                        nc.vector.tensor_tensor(ctxT_sb[off:off + DH, th, :],
                                                ctxT_sb[off:off + DH, th, :],
                                                r_ps[0:DH, :NQ], ALU.mult)

                # ---- out-proj + residual + LN1 --------------------------
                xres_sb = xrp.tile([P, KD, NQ], F32, tag="xres")
                for m in range(KD):
                    ps = mmp.tile([P, 512], F32, tag="mm")
                    for kc in range(KD):
                        nc.tensor.matmul(ps[:, :NQ], outW_sb[:, kc, m * P:(m + 1) * P],
                                         ctxT_sb[:, kc, :], start=(kc == 0), stop=(kc == KD - 1))
                    t_sb = sml.tile([P, NQ], F32, tag="rs_lnt", name="t_sb")
                    nc.vector.tensor_scalar_add(t_sb[:], ps[:, :NQ], outb_c[:, m:m + 1])
                    nc.vector.tensor_tensor(xres_sb[:, m, :], t_sb[:],
                                            gnTf_sb[:, m, :], ALU.add)
                layer_norm(xres_sb, ln1g_c, ln1b_c, xres_sb)
                xbf_sb = qhp.tile([P, KD, NQ], BF16, tag="qh", name="xbf_sb")
                for m in range(KD):
                    nc.vector.tensor_copy(xbf_sb[:, m, :], xres_sb[:, m, :])

                # ---- FFN: y = leaky_relu(x @ d1W + d1b) + x, then LN2 ----
                y_sb = yyp.tile([P, KD, NQ], F32, tag="yy")
                for m in range(KD):
                    ps = mmp.tile([P, 512], F32, tag="mm")
                    for kc in range(KD):
                        nc.tensor.matmul(ps[:, :NQ], d1W_sb[:, kc, m * P:(m + 1) * P],
                                         xbf_sb[:, kc, :], start=(kc == 0), stop=(kc == KD - 1))
                    t_sb = sml.tile([P, NQ], F32, tag="rs_lnt", name="t_sb")
                    nc.scalar.activation(t_sb[:], ps[:, :NQ], AF.Lrelu,
                                         bias=d1b_c[:, m:m + 1], alpha=0.01)
                    nc.vector.tensor_tensor(y_sb[:, m, :], t_sb[:],
                                            xres_sb[:, m, :], ALU.add)
                z_sb = zzp.tile([P, KD, NQ], F32, tag="zz")
                layer_norm(y_sb, ln2g_c, ln2b_c, z_sb)

                # ---- transpose back to [tok, feat] and store ------------
                out_sb = otp.tile([P, MQ, D], F32, tag="outp")
                for t in range(MQ):
                    for m in range(KD):
                        tr_ps = mmp.tile([P, 512], F32, tag="mm")
                        nc.tensor.transpose(tr_ps[:, :P], z_sb[:, m, t * P:(t + 1) * P], id_sb[:])
                        nc.vector.tensor_copy(out_sb[:, t, m * P:(m + 1) * P], tr_ps[:, :P])
                    nc.sync.dma_start(out[b, t * P:(t + 1) * P, :], out_sb[:, t, :])

    nc.compile()
    return nc


def kernel(**inputs):
    gn = np.asarray(inputs["graph_nodes"], dtype=np.float32)
    cond = np.asarray(inputs["conditioning_vector"], dtype=np.float32)
    mask = np.asarray(inputs["conditioning_attention_mask"])
    g = lambda k: np.asarray(inputs[k], dtype=np.float32)

    qW, qb = g("qW"), g("qb")
    kW, kb = g("kW"), g("kb")
    vW, vb = g("vW"), g("vb")
    in_qW, in_qb = g("in_qW"), g("in_qb")
    in_kW, in_kb = g("in_kW"), g("in_kb")
    in_vW, in_vb = g("in_vW"), g("in_vb")
    outW, outb = g("outW"), g("outb")
    ln1g, ln1b = g("ln1g"), g("ln1b")
    d1W, d1b = g("d1W"), g("d1b")
    ln2g, ln2b = g("ln2g"), g("ln2b")

    scale = 1.0 / np.sqrt(np.float32(DH))
    qWe = (qW @ in_qW) * scale
    qbe = (qb @ in_qW + in_qb) * scale
    kWe = kW @ in_kW
    kbe = kb @ in_kW + in_kb
    vWe = vW @ in_vW
    vbe = vb @ in_vW + in_vb

    # vWe augmented with a zero column per head; bias row carries vbe + ones
    vWe_aug = np.zeros((L, VA), np.float32)
    vbe_aug = np.zeros((VA,), np.float32)
    for h in range(H):
        vWe_aug[:, h * (DH + 1):h * (DH + 1) + DH] = vWe[:, h * DH:(h + 1) * DH]
        vbe_aug[h * (DH + 1):h * (DH + 1) + DH] = vbe[h * DH:(h + 1) * DH]
        vbe_aug[h * (DH + 1) + DH] = 1.0

    col = lambda v: np.ascontiguousarray(v.reshape(KD, P).T, dtype=np.float32)  # [P, KD]
    bcols = np.concatenate(
        [col(qbe), col(kbe), col(outb), col(d1b),
         col(ln1g), col(ln1b), col(ln2g), col(ln2b)], axis=1)

    valid01 = np.where(mask, 0.0, 1.0).astype(np.float32)  # [B, NK]

    key = "nc"
    if key not in _NC_CACHE:
        _NC_CACHE[key] = _build_nc()
    nc = _NC_CACHE[key]

    bf = lambda a: np.ascontiguousarray(a.astype(NPBF))
    shared = {
        "qWe": bf(qWe), "kWe": bf(kWe), "vWe": bf(vWe_aug),
        "outW": bf(outW), "d1W": bf(d1W),
        "bcols": np.ascontiguousarray(bcols),
        "vber": bf(vbe_aug[None, :]),
        "onesr": np.ones((1, P), np.float32),
        "onesrb": np.ones((1, P), NPBF),
        "onescb": np.ones((P, 1), NPBF),
        "onescf": np.ones((P, 1), np.float32),
        "ident": np.eye(P, dtype=np.float32),
    }
    in_maps = []
    for c in range(NCORES):
        bs = slice(c * BL, (c + 1) * BL)
        vp = np.zeros((P, BL * MK), np.float32)
        for i, bb in enumerate(range(c * BL, (c + 1) * BL)):
            vp[:, i * MK:(i + 1) * MK] = valid01[bb].reshape(MK, P).T
        in_maps.append({
            **shared,
            "gnT": bf(gn[bs].transpose(0, 2, 1)),
            "gnTf": np.ascontiguousarray(gn[bs].transpose(0, 2, 1)),
            "condT": bf(cond[bs].transpose(0, 2, 1)),
            "vld": vp,
        })

    res = run_bass_kernel_spmd(nc, in_maps, list(range(NCORES)))
    return np.concatenate([res.results[c]["out"] for c in range(NCORES)], axis=0)



# revision 16
# speedup vs baseline: 1.4366x; 1.4366x over previous
"""Trainium2 Bass kernel for nn_CrossAttentionGraphBlock (v2, fp8).

Strategy (hardcoded for B=16, NQ=512, NK=1024, D=768, L=512, H=12, DH=64):
 - Data-parallel over batch: 8 cores x 2 batches each. No collectives.
 - Host fuses outer q/k/v projections with MHA in-proj, folds the softmax
   scale and fp8-friendly scale factors into the weights, and pre-quantizes
   weights/activations to fp8e4m3.
 - All big matmuls run fp8 DoubleRow (2 contraction subtiles per
   instruction): q/k/v projections, P@V (ctx), out-proj.  S = K^T@Q stays
   fp8 non-DR (contraction 64).  FFN d1 stays fp16 for precision.
 - exp is fused on the Act engine: P' = exp(S'/(SQ*SK) + ln(CP)) written
   directly as fp8; the softmax denominator comes from an extra ones-column
   per head in the augmented V layout (masked rows zeroed), and the divide
   is folded into a per-partition scaled copy of the token-major ctx.
 - ctx is computed token-major (stationary = P^T tiles) then transposed
   back to feature-major in fp8 on the PE (cheap).
 - LayerNorm: stats via ones-column matmuls on fp16 copies; rsqrt as
   exp(-0.5*ln(var)) so one activation table serves the whole kernel;
   row broadcasts via gpsimd partition_broadcast; normalize as fp16
   tensor ops; ln1 gain folded into d1W host-side.
 - Software pipelining: the post-attention tail of batch b-1 is emitted
   between S/exp(b) and ctx(b) so the PE stays busy while Act runs exp.
 - Output written bf16 (host converts to fp32).
"""

import numpy as np
import ml_dtypes

import concourse.bass as bass
import concourse.mybir as mybir
import concourse.tile as tile
from concourse import bacc
from concourse.bass_utils import run_bass_kernel_spmd

P = 128
B, NQ, NK, D, L, H = 16, 512, 1024, 768, 512, 12
DH = D // H          # 64
NCORES = 8
BL = B // NCORES     # 2 batches per core
EPS = 1e-5
F32 = mybir.dt.float32
BF16 = mybir.dt.bfloat16
F16 = mybir.dt.float16
F8 = mybir.dt.float8e4
NPBF = ml_dtypes.bfloat16
NPF8 = ml_dtypes.float8_e4m3
AF = mybir.ActivationFunctionType
ALU = mybir.AluOpType
DR = mybir.MatmulPerfMode.DoubleRow

KD = D // P          # 6 chunks of D
KL = L // P          # 4 chunks of L
MQ = NQ // P         # 4 query-token tiles
MK = NK // P         # 8 key-token tiles
VA = H * (DH + 1)    # 780: per-head [64 v-cols | 1 ones-col]
HG = 3               # head groups for ctx
HPG = H // HG        # 4 heads per group

# fp8 scale factors (folded host-side; exp scale/bias compensates)
SQ, SK, SV, CP, CC, SWo = 256.0, 64.0, 64.0, 64.0, 64.0, 32.0
EXPSCALE = 1.0 / (SQ * SK)
EXPBIAS = float(np.log(CP))
OSCALE = 1.0 / (CC * SWo)
RCSCALE = CC / SV

_NC_CACHE = {}


def _build_nc(flags):
    has_qb, has_kb, has_vb, has_ob, has_d1b, has_l1b, has_l2b = flags
    nc = bacc.Bacc()

    gnT8 = nc.declare_dram_parameter("gnT8", [BL, D, NQ], F8, isOutput=False)
    condT8 = nc.declare_dram_parameter("condT8", [BL, L, NK], F8, isOutput=False)
    gnbf = nc.declare_dram_parameter("gnbf", [BL, D, NQ], BF16, isOutput=False)
    qWe8 = nc.declare_dram_parameter("qWe8", [D, D], F8, isOutput=False)
    kWe8 = nc.declare_dram_parameter("kWe8", [L, D], F8, isOutput=False)
    vWe8 = nc.declare_dram_parameter("vWe8", [L, VA], F8, isOutput=False)
    outW8 = nc.declare_dram_parameter("outW8", [D, D], F8, isOutput=False)
    d1Wh = nc.declare_dram_parameter("d1Wh", [D, D], F16, isOutput=False)
    bcols = nc.declare_dram_parameter("bcols", [P, 8 * KD], F32, isOutput=False)
    vld = nc.declare_dram_parameter("vld", [P, BL * MK], F32, isOutput=False)
    id8 = nc.declare_dram_parameter("id8", [P, P], F8, isOutput=False)
    idb = nc.declare_dram_parameter("idb", [P, P], BF16, isOutput=False)
    onesh = nc.declare_dram_parameter("onesh", [P, 1], F16, isOutput=False)
    onesr8 = nc.declare_dram_parameter("onesr8", [1, P], F8, isOutput=False)
    vber8 = nc.declare_dram_parameter("vber8", [1, VA], F8, isOutput=False)
    onesrb = nc.declare_dram_parameter("onesrb", [1, NQ], BF16, isOutput=False)
    outbr = nc.declare_dram_parameter("outbr", [1, D], BF16, isOutput=False)
    out = nc.declare_dram_parameter("out", [BL, NQ, D], BF16, isOutput=True)

    with tile.TileContext(nc) as tc:
        with (
            tc.tile_pool(name="const", bufs=1) as cst,
            tc.tile_pool(name="gin", bufs=2) as ginp,      # gnT8 / condT8 / gnbf
            tc.tile_pool(name="qk", bufs=2) as qkp,        # qhT8 / khT8 / vh8
            tc.tile_pool(name="pt", bufs=2) as ptp,        # pT8 head-group tiles
            tc.tile_pool(name="act", bufs=1) as acp,       # ctx8/ctxT8/xh/t1/...
            tc.tile_pool(name="sq", bufs=2) as sqp,        # squares / lrelu tmp
            tc.tile_pool(name="row", bufs=2) as rwp,       # [1,*] rows + bcasts
            tc.tile_pool(name="psA", bufs=2, space="PSUM") as psA,   # [P,1024] f32
            tc.tile_pool(name="psB", bufs=4, space="PSUM") as psB,   # [P,512] f32
        ):
            # ---- resident constants ------------------------------------
            qWe_sb = cst.tile([P, KD, D], F8, tag="qWe")
            nc.sync.dma_start(qWe_sb[:], qWe8[:].rearrange("(ko p) n -> p ko n", p=P))
            kWe_sb = cst.tile([P, KL, D], F8, tag="kWe")
            nc.sync.dma_start(kWe_sb[:], kWe8[:].rearrange("(ko p) n -> p ko n", p=P))
            vWe_sb = cst.tile([P, KL, VA], F8, tag="vWe")
            nc.sync.dma_start(vWe_sb[:], vWe8[:].rearrange("(ko p) n -> p ko n", p=P))
            outW_sb = cst.tile([P, KD, D], F8, tag="outW")
            nc.sync.dma_start(outW_sb[:], outW8[:].rearrange("(ko p) n -> p ko n", p=P))
            d1W_sb = cst.tile([P, KD, D], F16, tag="d1W")
            nc.sync.dma_start(d1W_sb[:], d1Wh[:].rearrange("(ko p) n -> p ko n", p=P))
            bc_sb = cst.tile([P, 8 * KD], F32, tag="bcols")
            nc.sync.dma_start(bc_sb[:], bcols[:])
            vld_sb = cst.tile([P, BL * MK], F32, tag="vld")
            nc.sync.dma_start(vld_sb[:], vld[:])
            id8_sb = cst.tile([P, P], F8, tag="id8")
            nc.sync.dma_start(id8_sb[:], id8[:])
            idb_sb = cst.tile([P, P], BF16, tag="idb")
            nc.sync.dma_start(idb_sb[:], idb[:])
            onesh_sb = cst.tile([P, 1], F16, tag="onesh")
            nc.sync.dma_start(onesh_sb[:], onesh[:])
            if has_vb:
                onesr8_sb = cst.tile([1, P], F8, tag="onesr8")
                nc.sync.dma_start(onesr8_sb[:], onesr8[:])
                vber8_sb = cst.tile([1, VA], F8, tag="vber8")
                nc.sync.dma_start(vber8_sb[:], vber8[:])
            if has_ob:
                onesrb_sb = cst.tile([1, NQ], BF16, tag="onesrb")
                nc.sync.dma_start(onesrb_sb[:], onesrb[:])
                outbr_sb = cst.tile([1, D], BF16, tag="outbr")
                nc.sync.dma_start(outbr_sb[:], outbr[:])

            qbe_c = bc_sb[:, 0 * KD:1 * KD]
            kbe_c = bc_sb[:, 1 * KD:2 * KD]
            d1b_c = bc_sb[:, 2 * KD:3 * KD]
            g1_c = bc_sb[:, 3 * KD:4 * KD]
            g2_c = bc_sb[:, 4 * KD:5 * KD]
            b1_c = bc_sb[:, 5 * KD:6 * KD]
            b2_c = bc_sb[:, 6 * KD:7 * KD]
            expb_c = bc_sb[:, 7 * KD:7 * KD + 1]      # ln(CP) on all partitions
            zero_c = bc_sb[:, 7 * KD + 1:7 * KD + 2]  # 0.0

            def ln_rows(x16, st1, st2, tag):
                """Feature-major LN stats -> (rs_bc, u_bc) fp16 [P, NQ] in SBUF.
                x16: fp16 [P, KD, NQ]; st1/st2: psB tiles (row 0 used)."""
                sq_sb = sqp.tile([P, KD, NQ], F16, tag="sq", name=f"sq_{tag}")
                for c in range(KD):
                    nc.tensor.matmul(st1[0:1, :], onesh_sb[:], x16[:, c, :],
                                     start=(c == 0), stop=(c == KD - 1))
                    nc.gpsimd.tensor_tensor(sq_sb[:, c, :], x16[:, c, :],
                                            x16[:, c, :], ALU.mult)
                    nc.tensor.matmul(st2[0:1, :], onesh_sb[:], sq_sb[:, c, :],
                                     start=(c == 0), stop=(c == KD - 1))
                m_row = rwp.tile([1, NQ], F32, tag="m_row", name=f"m_{tag}")
                nc.vector.tensor_scalar_mul(m_row[:], st1[0:1, :], 1.0 / D)
                var_row = rwp.tile([1, NQ], F32, tag="var_row", name=f"v_{tag}")
                nc.vector.tensor_scalar(var_row[:], st2[0:1, :], 1.0 / D, EPS,
                                        ALU.mult, ALU.add)
                msq_row = rwp.tile([1, NQ], F32, tag="msq_row", name=f"ms_{tag}")
                nc.vector.tensor_tensor(msq_row[:], m_row[:], m_row[:], ALU.mult)
                nc.vector.tensor_tensor(var_row[:], var_row[:], msq_row[:], ALU.subtract)
                lnv_row = rwp.tile([1, NQ], F32, tag="lnv_row", name=f"lv_{tag}")
                nc.scalar.activation(lnv_row[:], var_row[:], AF.Ln, bias=zero_c[0:1, :])
                rs_row = rwp.tile([1, NQ], F16, tag="rs_row", name=f"rs_{tag}")
                nc.scalar.activation(rs_row[:], lnv_row[:], AF.Exp, scale=-0.5,
                                     bias=zero_c[0:1, :])
                u_row = rwp.tile([1, NQ], F16, tag="u_row", name=f"u_{tag}")
                nc.vector.scalar_tensor_tensor(u_row[:], m_row[:], -1.0, rs_row[:],
                                               ALU.mult, ALU.mult)
                rs_bc = rwp.tile([P, NQ], F16, tag="rs_bc", name=f"rsb_{tag}")
                nc.gpsimd.partition_broadcast(rs_bc[:], rs_row[:])
                u_bc = rwp.tile([P, NQ], F16, tag="u_bc", name=f"ub_{tag}")
                nc.gpsimd.partition_broadcast(u_bc[:], u_row[:])
                return rs_bc, u_bc

            state = {}

            def emit_proj(b):
                gnT_sb = ginp.tile([P, KD, NQ], F8, tag="gnT")
                nc.sync.dma_start(gnT_sb[:], gnT8[b].rearrange("(ko p) t -> p ko t", p=P))
                condT_sb = ginp.tile([P, KL, NK], F8, tag="condT")
                nc.sync.dma_start(condT_sb[:], condT8[b].rearrange("(ko p) t -> p ko t", p=P))
                gnb_sb = ginp.tile([P, KD, NQ], BF16, tag="gnbf")
                nc.sync.dma_start(gnb_sb[:], gnbf[b].rearrange("(ko p) t -> p ko t", p=P))

                # q proj: fp8 DR, contraction D=768 (3 pairs)
                qhT = qkp.tile([P, KD, NQ], F8, tag="qh")
                for m in range(KD):
                    ps = psB.tile([P, NQ], F32, tag="psB", name="q_ps")
                    for j in range(KD // 2):
                        nc.tensor.matmul(ps[:], qWe_sb[:, 2 * j:2 * j + 2, m * P:(m + 1) * P],
                                         gnT_sb[:, 2 * j:2 * j + 2, :],
                                         start=(j == 0), stop=(j == KD // 2 - 1),
                                         perf_mode=DR)
                    if has_qb:
                        nc.vector.tensor_scalar_add(qhT[:, m, :], ps[:], qbe_c[:, m:m + 1])
                    else:
                        nc.vector.tensor_copy(qhT[:, m, :], ps[:])

                # k proj: fp8 DR, contraction L=512 (2 pairs), out [P, NK]
                khT = qkp.tile([P, KD, NK], F8, tag="kh")
                for m in range(KD):
                    ps = psA.tile([P, NK], F32, tag="psA", name="k_ps")
                    for half in range(2):
                        for j in range(KL // 2):
                            nc.tensor.matmul(ps[:, half * NQ:(half + 1) * NQ],
                                             kWe_sb[:, 2 * j:2 * j + 2, m * P:(m + 1) * P],
                                             condT_sb[:, 2 * j:2 * j + 2, half * NQ:(half + 1) * NQ],
                                             start=(j == 0), stop=(j == KL // 2 - 1),
                                             perf_mode=DR)
                    if has_kb:
                        nc.vector.tensor_scalar_add(khT[:, m, :], ps[:], kbe_c[:, m:m + 1])
                    else:
                        nc.vector.tensor_copy(khT[:, m, :], ps[:])

                # v proj: fp8 DR; vh8 [P, MK, H, DH+1] (ones-col per head)
                vh8 = qkp.tile([P, MK, H, DH + 1], F8, tag="vh")
                for mk in range(MK):
                    ps = psA.tile([P, NK], F32, tag="psA", name="v_ps")
                    for (n0, nw) in ((0, 390), (390, 390)):
                        for j in range(KL // 2):
                            nc.tensor.matmul(ps[:, n0:n0 + nw],
                                             condT_sb[:, 2 * j:2 * j + 2, mk * P:(mk + 1) * P],
                                             vWe_sb[:, 2 * j:2 * j + 2, n0:n0 + nw],
                                             start=(j == 0),
                                             stop=(has_vb is False and j == KL // 2 - 1),
                                             perf_mode=DR)
                        if has_vb:
                            nc.tensor.matmul(ps[:, n0:n0 + nw], onesr8_sb[:],
                                             vber8_sb[:, n0:n0 + nw], start=False, stop=True)
                    vcol = vld_sb[:, b * MK + mk:b * MK + mk + 1]
                    nc.vector.tensor_scalar_mul(
                        vh8[:, mk, :, :].rearrange("p h d -> p (h d)"), ps[:, 0:VA], vcol)
                    nc.gpsimd.tensor_copy(vh8[:, mk, :, DH],
                                          vcol.to_broadcast([P, H]))
                state[b] = dict(qhT=qhT, khT=khT, vh8=vh8, gnb=gnb_sb)

            def emit_sexp_group(b, grp):
                """S + exp for heads [grp*HPG, (grp+1)*HPG); returns pT8 tile."""
                qhT, khT = state[b]["qhT"], state[b]["khT"]
                pT8 = ptp.tile([P, HPG, MK, NQ], F8, tag="pt", name=f"pT8_{grp}")
                for hh in range(HPG):
                    h = grp * HPG + hh
                    cs, off = h // 2, DH * (h % 2)
                    for g in range(MK // 2):
                        ps = psA.tile([P, NK], F32, tag="psA", name="s_ps")
                        for half in range(2):
                            mk = 2 * g + half
                            nc.tensor.matmul(ps[:, half * NQ:(half + 1) * NQ],
                                             khT[off:off + DH, cs, mk * P:(mk + 1) * P],
                                             qhT[off:off + DH, cs, :],
                                             start=True, stop=True)
                        nc.scalar.activation(pT8[:, hh, 2 * g:2 * g + 2, :], ps[:],
                                             AF.Exp, scale=EXPSCALE, bias=expb_c[:])
                state[b][f"pT8_{grp}"] = pT8

            def emit_ctx(b):
                """Token-major ctx per (qtile, head-group) + fp8 transpose back."""
                vh8 = state[b]["vh8"]
                ctx8 = acp.tile([P, MQ, D], BF16, tag="ctx8")
                for grp in range(HG):
                    pT8 = state[b][f"pT8_{grp}"]
                    for t in range(MQ):
                        cps = psB.tile([P, HPG, DH + 1], F32, tag="psB", name="c_ps")
                        for hh in range(HPG):
                            h = grp * HPG + hh
                            for j in range(MK // 2):
                                nc.tensor.matmul(cps[:, hh, :],
                                                 pT8[:, hh, 2 * j:2 * j + 2, t * P:(t + 1) * P],
                                                 vh8[:, 2 * j:2 * j + 2, h, :],
                                                 start=(j == 0), stop=(j == MK // 2 - 1),
                                                 perf_mode=DR)
                        rcol = rwp.tile([P, HPG], F32, tag="rcol", name="rcol")
                        nc.vector.reciprocal(rcol[:], cps[:, :, DH])
                        nc.vector.scalar_tensor_tensor(
                            ctx8[:, t, grp * HPG * DH:(grp + 1) * HPG * DH]
                                .rearrange("p (h d) -> p h d", d=DH),
                            cps[:, :, 0:DH], RCSCALE,
                            rcol[:, :, None].to_broadcast([P, HPG, DH]),
                            ALU.mult, ALU.mult)
                # transpose back to feature-major fp8
                ctxT8 = acp.tile([P, KD, NQ], F8, tag="ctxT8")
                for c in range(KD):
                    tp = psB.tile([P, NQ], BF16, tag="psB", name="t_ps")
                    for t in range(MQ):
                        nc.tensor.transpose(tp[:, t * P:(t + 1) * P],
                                            ctx8[:, t, c * P:(c + 1) * P], idb_sb[:])
                    nc.vector.tensor_copy(ctxT8[:, c, :], tp[:])
                state[b]["ctxT8"] = ctxT8

            def emit_tail(b):
                ctxT8, gnb = state[b]["ctxT8"], state[b]["gnb"]
                # out-proj fp8 DR + residual -> xh fp16
                xh = acp.tile([P, KD, NQ], F16, tag="xh")
                for m in range(KD):
                    ps = psB.tile([P, NQ], F32, tag="psB", name="o_ps")
                    for j in range(KD // 2):
                        nc.tensor.matmul(ps[:], outW_sb[:, 2 * j:2 * j + 2, m * P:(m + 1) * P],
                                         ctxT8[:, 2 * j:2 * j + 2, :],
                                         start=(j == 0), stop=(not has_ob and j == KD // 2 - 1),
                                         perf_mode=DR)
                    if has_ob:
                        nc.tensor.matmul(ps[:], outbr_sb[:, m * P:(m + 1) * P],
                                         onesrb_sb[:], start=False, stop=True)
                    nc.vector.scalar_tensor_tensor(xh[:, m, :], ps[:], OSCALE,
                                                   gnb[:, m, :], ALU.mult, ALU.add)
                # LN1 core
                st1 = psB.tile([P, NQ], F32, tag="psB", name="st1")
                st2 = psB.tile([P, NQ], F32, tag="psB", name="st2")
                rs_bc, u_bc = ln_rows(xh, st1, st2, f"l1_{b}")
                t1 = acp.tile([P, KD, NQ], F16, tag="t1")
                for c in range(KD):
                    tmp = sqp.tile([P, NQ], F16, tag="ntmp", name="ntmp")
                    nc.vector.tensor_tensor(tmp[:], xh[:, c, :], rs_bc[:], ALU.mult)
                    nc.vector.tensor_tensor(t1[:, c, :], tmp[:], u_bc[:], ALU.add)
                # FFN: d1 fp16 (g folded) + lrelu + residual (g1 applied here)
                y16 = acp.tile([P, KD, NQ], F16, tag="y16")
                for m in range(KD):
                    ps = psB.tile([P, NQ], F32, tag="psB", name="d_ps")
                    for c in range(KD):
                        nc.tensor.matmul(ps[:], d1W_sb[:, c, m * P:(m + 1) * P],
                                         t1[:, c, :], start=(c == 0), stop=(c == KD - 1))
                    lr = sqp.tile([P, NQ], F16, tag="ntmp", name="lr")
                    nc.scalar.activation(lr[:], ps[:], AF.Lrelu,
                                         bias=d1b_c[:, m:m + 1], alpha=0.01)
                    nc.vector.scalar_tensor_tensor(y16[:, m, :], t1[:, m, :],
                                                   g1_c[:, m:m + 1], lr[:],
                                                   ALU.mult, ALU.add)
                    if has_l1b:
                        nc.vector.tensor_scalar_add(y16[:, m, :], y16[:, m, :],
                                                    b1_c[:, m:m + 1])
                # LN2 -> z bf16
                st1b = psB.tile([P, NQ], F32, tag="psB", name="st1b")
                st2b = psB.tile([P, NQ], F32, tag="psB", name="st2b")
                rs2_bc, u2_bc = ln_rows(y16, st1b, st2b, f"l2_{b}")
                z = acp.tile([P, KD, NQ], BF16, tag="z")
                for c in range(KD):
                    tmp = sqp.tile([P, NQ], F16, tag="ntmp", name="ztmp")
                    nc.vector.scalar_tensor_tensor(tmp[:], y16[:, c, :],
                                                   g2_c[:, c:c + 1], rs2_bc[:],
                                                   ALU.mult, ALU.mult)
                    nc.vector.scalar_tensor_tensor(z[:, c, :], u2_bc[:],
                                                   g2_c[:, c:c + 1], tmp[:],
                                                   ALU.mult, ALU.add)
                    if has_l2b:
                        nc.vector.tensor_scalar_add(z[:, c, :], z[:, c, :],
                                                    b2_c[:, c:c + 1])
                # transpose to token-major and store (bf16)
                out_sb = acp.tile([P, MQ, D], BF16, tag="out_sb")
                for t in range(MQ):
                    fp = psB.tile([P, D], BF16, tag="psB", name="f_ps")
                    for c in range(KD):
                        nc.tensor.transpose(fp[:, c * P:(c + 1) * P],
                                            z[:, c, t * P:(t + 1) * P], idb_sb[:])
                    nc.vector.tensor_copy(out_sb[:, t, :], fp[:])
                    nc.sync.dma_start(out[b, t * P:(t + 1) * P, :], out_sb[:, t, :])

            # ---- main software-pipelined loop ---------------------------
            for b in range(BL):
                emit_proj(b)
                for grp in range(HG):
                    emit_sexp_group(b, grp)
                if b > 0:
                    emit_tail(b - 1)
                emit_ctx(b)
            emit_tail(BL - 1)

    nc.compile()
    return nc


def kernel(**inputs):
    gn = np.asarray(inputs["graph_nodes"], dtype=np.float32)
    cond = np.asarray(inputs["conditioning_vector"], dtype=np.float32)
    mask = np.asarray(inputs["conditioning_attention_mask"])
    g = lambda k: np.asarray(inputs[k], dtype=np.float32)

    qW, qb = g("qW"), g("qb")
    kW, kb = g("kW"), g("kb")
    vW, vb = g("vW"), g("vb")
    in_qW, in_qb = g("in_qW"), g("in_qb")
    in_kW, in_kb = g("in_kW"), g("in_kb")
    in_vW, in_vb = g("in_vW"), g("in_vb")
    outW, outb = g("outW"), g("outb")
    ln1g, ln1b = g("ln1g"), g("ln1b")
    d1W, d1b = g("d1W"), g("d1b")
    ln2g, ln2b = g("ln2g"), g("ln2b")

    scale = 1.0 / np.sqrt(np.float32(DH))
    qWe = (qW @ in_qW) * scale
    qbe = ((qb @ in_qW + in_qb) * scale * SQ).astype(np.float32)
    kWe = kW @ in_kW
    kbe = (kb @ in_kW + in_kb) * SK
    vWe = vW @ in_vW
    vbe = (vb @ in_vW + in_vb) * SV

    # augmented V: per head [64 cols | ones-col] (ones written on device)
    vWe_aug = np.zeros((L, VA), np.float32)
    vbe_aug = np.zeros((VA,), np.float32)
    for h in range(H):
        vWe_aug[:, h * (DH + 1):h * (DH + 1) + DH] = vWe[:, h * DH:(h + 1) * DH] * SV
        vbe_aug[h * (DH + 1):h * (DH + 1) + DH] = vbe[h * DH:(h + 1) * DH]

    d1Wg = ln1g[:, None] * d1W          # fold LN1 gain into d1W
    d1be = d1b + ln1b @ d1W             # fold LN1 bias into d1 bias

    col = lambda v: np.ascontiguousarray(v.reshape(KD, P).T, dtype=np.float32)
    lastcols = np.zeros((P, KD), np.float32)
    lastcols[:, 0] = EXPBIAS
    bcols = np.concatenate(
        [col(qbe), col(kbe), col(d1be), col(ln1g), col(ln2g), col(ln1b),
         col(ln2b), lastcols], axis=1)

    valid01 = np.where(mask, 0.0, 1.0).astype(np.float32)  # [B, NK]

    flags = (bool(np.any(qbe)), bool(np.any(kbe)), bool(np.any(vbe_aug)),
             bool(np.any(outb)), bool(np.any(d1be)), bool(np.any(ln1b)),
             bool(np.any(ln2b)))
    key = ("nc",) + flags
    if key not in _NC_CACHE:
        _NC_CACHE.clear()
        _NC_CACHE[key] = _build_nc(flags)
        _NC_CACHE["nc"] = _NC_CACHE[key]
    nc = _NC_CACHE[key]

    f8 = lambda a: np.ascontiguousarray(a.astype(NPF8))
    bf = lambda a: np.ascontiguousarray(a.astype(NPBF))
    shared = {
        "qWe8": f8(qWe * SQ), "kWe8": f8(kWe * SK), "vWe8": f8(vWe_aug),
        "outW8": f8(outW * SWo), "d1Wh": np.ascontiguousarray(d1Wg.astype(np.float16)),
        "bcols": np.ascontiguousarray(bcols),
        "id8": f8(np.eye(P, dtype=np.float32)),
        "idb": bf(np.eye(P, dtype=np.float32)),
        "onesh": np.ones((P, 1), np.float16),
        "onesr8": f8(np.ones((1, P), np.float32)),
        "vber8": f8(vbe_aug[None, :]),
        "onesrb": bf(np.ones((1, NQ), np.float32)),
        "outbr": bf((outb * CC * SWo)[None, :]),
    }
    in_maps = []
    for c in range(NCORES):
        bs = slice(c * BL, (c + 1) * BL)
        vp = np.zeros((P, BL * MK), np.float32)
        for i, bb in enumerate(range(c * BL, (c + 1) * BL)):
            vp[:, i * MK:(i + 1) * MK] = valid01[bb].reshape(MK, P).T
        in_maps.append({
            **shared,
            "gnT8": f8(gn[bs].transpose(0, 2, 1)),
            "condT8": f8(cond[bs].transpose(0, 2, 1)),
            "gnbf": bf(gn[bs].transpose(0, 2, 1)),
            "vld": vp,
        })

    res = run_bass_kernel_spmd(nc, in_maps, list(range(NCORES)))
    outs = [np.asarray(res.results[c]["out"]).astype(np.float32) for c in range(NCORES)]
    return np.concatenate(outs, axis=0)
